# revision 8
# baseline (speedup 1.0000x reference)
"""Multi-head attention + residual + LayerNorm kernel for Trainium2 (8 NeuronCores).

Sharding: pure data parallel over batch (B=8 -> 1 batch element per core).
No collectives.

Key restructuring vs a direct implementation: per head, the QK and VO weight
pairs are folded on the host into single [D, D] matrices
    Wqk_h = Wq_h @ Wk_h^T   (scores = Q Wqk K^T, scale folded into exp)
    Wvo_h = Wv_h @ Wo_h     (out   = (attn @ V) Wvo)
which removes the K and V projections entirely and halves the contraction
depth of the scores and attention*V matmuls (512 instead of 1024). All heavy
matmuls run fp8e4 DoubleRow (K packed in pairs -> [128, 2, free] tiles) with
fp32 PSUM accumulation. The folded weights are scaled x64 (fp8 subnormal
avoidance); exp() folds in a x1/4 bias so E and E@V stay inside fp8e4m3
range (alpha cancels in the softmax ratio). The attention mask is applied
additively INSIDE the scores PSUM accumulation via one extra DoubleRow
matmul per psum: diag(-240) stationary x {0,240} mask tile adds -57600 to
masked scores, so exp() emits exact fp8 zeros and no post-exp elementwise
mask op exists at all (it was the critical-path serializer).

Softmax normalization is folded into the attn@V eviction: the denominator
row (every partition identical, from a ones-matmul with value 2^-6) is
reciprocal'd once per head and multiplied in during the PSUM->fp8 eviction.
That makes per-head outputs summable, so the output projection accumulates
over ALL heads inside PSUM in a tail phase (one residual-add per row tile).

Per-core dataflow per head h (Sq = query positions, Sk = key positions):
  tT [Dc,Sq]   = Wqk_h^T @ Qb^T          (DR fp8; evict fp8 pairs, ACT)
  ST [Sk,Sq]   = Kb^T-chunks^T @ tT - 57600*mask   (DR fp8, transposed)
  E            = exp(ST/2048 + ln 1/4)   (ACT from PSUM, fp8 pair tiles)
  den[128,Sq]  = (2^-6 ones)^T @ E ; recip = 1/den   (DVE, from PSUM)
  cxv[Dc,Sq]   = (Vb^T-chunks^T @ E) * recip         (DR fp8 + DVE evict)
tail:
  outp[Sq,D]   = sum_h cxv_h^T @ Wvo_h   (PSUM accumulation over heads)
  acc          = outp/4096 + Q           (DVE STT)
  LayerNorm over D per row (bn_stats/bn_aggr + sqrt + reciprocal).

The per-head stages are software-pipelined (head h+1's tT projection issues
between head h's scores and attention tail) to keep the PE array busy.
"""

import sys

sys.path.insert(0, "/opt/trn_rl_repo")

import numpy as np
import ml_dtypes

B, S, D, H = 8, 1024, 512, 8
DH = 2 * D            # per-head dim (module uses d_model*2 per head)
P = 128               # SBUF partitions
NS = S // 512         # 512-wide free-dim chunks over sequence (2)
MT = S // P           # 128-partition tiles over sequence (8)
KD = D // P           # 128-chunks over d_model (4)
JD = KD // 2          # DoubleRow K-pairs over d_model (2)
JS = MT // 2          # DoubleRow K-pairs over sequence (4)
SQ = 64.0             # host scale on Wqk (fp8 subnormal avoidance)
SV = 64.0             # host scale on Wvo
OMEGA = 2.0 ** -6     # ones value in den matmul (min normal fp8) -> cxv = ctx/OMEGA
PSC = 1.0 / (SV / OMEGA)                # tail un-scale: 1/4096
ESC = 1.0 / (SQ * float(np.sqrt(DH)))   # exp input scale
EBIAS = float(np.log(0.25))             # exp bias: E *= 1/4 (fp8 range)

_cache = {}


def build_nc(repeat=1, nonzero_bias=False, nonzero_affine=False):
    """Build the per-core Bass program. All 8 cores run this SPMD."""
    import concourse.bass as bass
    import concourse.tile as tile
    from concourse import bacc, mybir

    f32 = mybir.dt.float32
    f8 = mybir.dt.float8e4
    AF = mybir.ActivationFunctionType
    OP = mybir.AluOpType
    DR = mybir.MatmulPerfMode.DoubleRow

    nc = bacc.Bacc("TRN2", target_bir_lowering=False, debug=False, num_devices=8)

    # DRAM I/O (per core). Pair layouts: [j, 128(p), 2(par), cols] where the
    # contraction index is k = 2*j + par, row = k*128 + p.
    xq_d = nc.dram_tensor("xq", [JD, P, 2, S], f8, kind="ExternalInput").ap()
    xk_d = nc.dram_tensor("xk", [JD, P, 2, S], f8, kind="ExternalInput").ap()
    xv_d = nc.dram_tensor("xv", [JS, P, 2, D], f8, kind="ExternalInput").ap()
    qres = nc.dram_tensor("qres", [S, D], f32, kind="ExternalInput").ap()
    maskM = nc.dram_tensor("maskM", [JS, P, 2, S], f8, kind="ExternalInput").ap()
    diag_d = nc.dram_tensor("diagm", [2, P, 2, P], f8, kind="ExternalInput").ap()
    wqk_d = nc.dram_tensor("wqk", [H, JD, P, 2, D], f8, kind="ExternalInput").ap()
    wvo_d = nc.dram_tensor("wvo", [H, JD, P, 2, D], f8, kind="ExternalInput").ap()
    if nonzero_bias:
        ebias_d = nc.dram_tensor("ebias", [H, S], f32, kind="ExternalInput").ap()
        boeff = nc.dram_tensor("boeff", [D], f32, kind="ExternalInput").ap()
    if nonzero_affine:
        gam = nc.dram_tensor("gam", [D], f32, kind="ExternalInput").ap()
        bet = nc.dram_tensor("bet", [D], f32, kind="ExternalInput").ap()
    out = nc.dram_tensor("out", [S, D], f32, kind="ExternalOutput").ap()

    def bcast_ap(src_1d, n):
        return bass.AP(tensor=src_1d.tensor, offset=src_1d.offset,
                       ap=[[0, P]] + list(src_1d.ap))

    with tile.TileContext(nc) as tc:
        import contextlib
        with contextlib.ExitStack() as ctx:
            const = ctx.enter_context(tc.tile_pool(name="const", bufs=1))
            persist = ctx.enter_context(tc.tile_pool(name="persist", bufs=1))
            wpool = ctx.enter_context(tc.tile_pool(name="wpool", bufs=4))
            wvopool = ctx.enter_context(tc.tile_pool(name="wvop", bufs=2 * H))
            tt_pool = ctx.enter_context(tc.tile_pool(name="tt", bufs=4))
            e_pool = ctx.enter_context(tc.tile_pool(name="ee", bufs=8))
            cx_pool = ctx.enter_context(tc.tile_pool(name="cx", bufs=2 * H))
            den_pool = ctx.enter_context(tc.tile_pool(name="den", bufs=2))
            resid_pool = ctx.enter_context(tc.tile_pool(name="resid", bufs=8))
            stat_pool = ctx.enter_context(tc.tile_pool(name="stat", bufs=8))
            mpool = ctx.enter_context(tc.tile_pool(name="mask", bufs=JS))
            mm_psum = ctx.enter_context(tc.tile_pool(name="mmps", bufs=3, space="PSUM"))
            sm_psum = ctx.enter_context(tc.tile_pool(name="smps", bufs=2, space="PSUM"))

            ones_pair = const.tile([P, 2, P], f8)
            nc.vector.memset(ones_pair, OMEGA)
            diag_par = [const.tile([P, 2, P], f8, tag=f"dg{i}", name=f"dg{i}")
                        for i in range(2)]
            for i in range(2):
                nc.sync.dma_start(diag_par[i], diag_d[i])
            eps_t = const.tile([P, 1], f32)
            nc.vector.memset(eps_t, 1e-5)
            ebias_t = const.tile([P, 1], f32)
            nc.vector.memset(ebias_t, EBIAS)
            psc_t = const.tile([P, 1], f32)
            nc.vector.memset(psc_t, PSC)

            if nonzero_bias:
                ebias_sb = const.tile([P, H * MT], f32)
                nc.sync.dma_start(ebias_sb, ebias_d.rearrange("h (c p) -> p (h c)", p=P))
                bo_b = const.tile([P, D], f32)
                nc.sync.dma_start(bo_b, bcast_ap(boeff, D))
            if nonzero_affine:
                gam_b = const.tile([P, D], f32)
                nc.sync.dma_start(gam_b, bcast_ap(gam, D))
                bet_b = const.tile([P, D], f32)
                nc.sync.dma_start(bet_b, bcast_ap(bet, D))

            # Persistent SBUF inputs (fp8 pair tiles)
            xq_sb = [persist.tile([P, 2, S], f8, tag=f"xq{j}", name=f"xq{j}") for j in range(JD)]
            xk_sb = [persist.tile([P, 2, S], f8, tag=f"xk{j}", name=f"xk{j}") for j in range(JD)]
            xv_sb = [persist.tile([P, 2, D], f8, tag=f"xv{j}", name=f"xv{j}") for j in range(JS)]

            def body(iv=None):
                wqk0 = [wpool.tile([P, 2, D], f8, tag="w", name="wqk0") for _ in range(JD)]
                for j in range(JD):
                    nc.sync.dma_start(xq_sb[j], xq_d[j])
                    nc.sync.dma_start(wqk0[j], wqk_d[0, j])
                for j in range(JD):
                    nc.sync.dma_start(xk_sb[j], xk_d[j])
                for j in range(JS):
                    nc.sync.dma_start(xv_sb[j], xv_d[j])
                mask_sb = [mpool.tile([P, 2, S], f8, tag="mk", name="mk") for _ in range(JS)]
                for j in range(JS):
                    nc.sync.dma_start(mask_sb[j], maskM[j])
                qres_sb = [resid_pool.tile([P, D], f32, tag="resid", name="qres_sb")
                           for _ in range(MT)]
                for m in range(MT):
                    nc.sync.dma_start(qres_sb[m], qres[m * P:(m + 1) * P, :])

                # ---- per-head stages (software-pipelined: A(h+1) issues
                # between S(h) and B(h) so the PE has independent work while
                # head h's exp/mask evictions drain)
                def stage_A(h, wqk_t):
                    """tT projection: [Dc, Sq] fp8 pairs."""
                    tt = [tt_pool.tile([P, 2, S], f8, tag="tt", name="tt") for _ in range(JD)]
                    for c in range(KD):
                        for n in range(NS):
                            ps = sm_psum.tile([P, 512], f32, tag="sm", name="pa")
                            for j in range(JD):
                                nc.tensor.matmul(
                                    ps,
                                    lhsT=wqk_t[j][:, :, c * P:(c + 1) * P],
                                    rhs=xq_sb[j][:, :, n * 512:(n + 1) * 512],
                                    start=(j == 0), stop=(j == JD - 1),
                                    perf_mode=DR, skip_group_check=True)
                            dst = tt[c // 2][:, c % 2, n * 512:(n + 1) * 512]
                            if c % 2 == 0:
                                nc.scalar.copy(dst, ps)
                            else:
                                nc.vector.tensor_copy(dst, ps)
                    return tt

                def stage_S(h, tt):
                    """Transposed scores (+additive mask) -> exp -> E fp8."""
                    est = [e_pool.tile([P, 2, S], f8, tag="e", name="est") for _ in range(JS)]
                    for m in range(MT):
                        ps = mm_psum.tile([P, S], f32, tag="mm", name="ps")
                        for j in range(JD):
                            for n in range(NS):
                                nc.tensor.matmul(
                                    ps[:, n * 512:(n + 1) * 512],
                                    lhsT=xk_sb[j][:, :, m * P:(m + 1) * P],
                                    rhs=tt[j][:, :, n * 512:(n + 1) * 512],
                                    start=(j == 0), stop=False,
                                    perf_mode=DR, skip_group_check=True)
                        for n in range(NS):
                            nc.tensor.matmul(
                                ps[:, n * 512:(n + 1) * 512],
                                lhsT=diag_par[m % 2],
                                rhs=mask_sb[m // 2][:, :, n * 512:(n + 1) * 512],
                                start=False, stop=(n == NS - 1),
                                perf_mode=DR, skip_group_check=True)
                        if nonzero_bias:
                            bcol = ebias_sb[:, h * MT + m:h * MT + m + 1]
                        else:
                            bcol = ebias_t
                        nc.scalar.activation(est[m // 2][:, m % 2, :], ps,
                                             AF.Exp, bias=bcol, scale=ESC)
                    return est

                def stage_B(h, est, cxv):
                    """den + recip, attn@V with fused normalization. The den
                    matmuls interleave into ctxVT chunk 0 so the est[3]
                    (exp m6/m7) consumers issue as late as possible."""
                    def ctx_mm(ps, c, j):
                        for n in range(NS):
                            nc.tensor.matmul(
                                ps[:, n * 512:(n + 1) * 512],
                                lhsT=xv_sb[j][:, :, c * P:(c + 1) * P],
                                rhs=est[j][:, :, n * 512:(n + 1) * 512],
                                start=(j == 0), stop=(j == JS - 1),
                                perf_mode=DR, skip_group_check=True)

                    ps0 = mm_psum.tile([P, S], f32, tag="mm", name="ps")
                    for j in range(JS - 1):
                        ctx_mm(ps0, 0, j)
                    psd = mm_psum.tile([P, S], f32, tag="mm", name="ps")
                    for j in range(JS):
                        for n in range(NS):
                            nc.tensor.matmul(
                                psd[:, n * 512:(n + 1) * 512], lhsT=ones_pair,
                                rhs=est[j][:, :, n * 512:(n + 1) * 512],
                                start=(j == 0), stop=(j == JS - 1),
                                perf_mode=DR, skip_group_check=True)
                    ctx_mm(ps0, 0, JS - 1)
                    recip = den_pool.tile([P, S], f32, tag="den", name="recip")
                    nc.vector.reciprocal(recip, psd)
                    nc.vector.tensor_tensor(out=cxv[0][:, 0, :],
                                            in0=ps0, in1=recip, op=OP.mult)

                    for c in range(1, KD):
                        ps = mm_psum.tile([P, S], f32, tag="mm", name="ps")
                        for j in range(JS):
                            ctx_mm(ps, c, j)
                        nc.vector.tensor_tensor(out=cxv[c // 2][:, c % 2, :],
                                                in0=ps, in1=recip, op=OP.mult)

                # pipeline: A(0), then per h: S(h), [prefetch + A(h+1)], B(h)
                wqk_t = wqk0
                cxv_all = []
                wvo_all = []
                tt = stage_A(0, wqk_t)
                for h in range(H):
                    wvo_t = [wvopool.tile([P, 2, D], f8, tag="wv", name="wvo_t")
                             for _ in range(JD)]
                    for j in range(JD):
                        nc.sync.dma_start(wvo_t[j], wvo_d[h, j])
                    wvo_all.append(wvo_t)
                    est = stage_S(h, tt)
                    if h + 1 < H:
                        wqk_n = [wpool.tile([P, 2, D], f8, tag="w", name="wqk_n")
                                 for _ in range(JD)]
                        for j in range(JD):
                            nc.sync.dma_start(wqk_n[j], wqk_d[h + 1, j])
                        tt = stage_A(h + 1, wqk_n)
                    cxv = [cx_pool.tile([P, 2, S], f8, tag="cx", name="cxv")
                           for _ in range(JD)]
                    cxv_all.append(cxv)
                    stage_B(h, est, cxv)

                # ---- tail: output projection accumulated over all heads,
                # then residual add + LayerNorm + store
                for m in range(MT):
                    ps2 = sm_psum.tile([P, D], f32, tag="sm", name="ops")
                    for h in range(H):
                        for j in range(JD):
                            nc.tensor.matmul(
                                ps2, lhsT=cxv_all[h][j][:, :, m * P:(m + 1) * P],
                                rhs=wvo_all[h][j],
                                start=(h == 0 and j == 0),
                                stop=(h == H - 1 and j == JD - 1),
                                perf_mode=DR, skip_group_check=True)
                    x = resid_pool.tile([P, D], f32, tag="acc", bufs=8, name="acc")
                    nc.vector.scalar_tensor_tensor(
                        out=x, in0=ps2, scalar=psc_t, in1=qres_sb[m],
                        op0=OP.mult, op1=OP.add)
                    if nonzero_bias:
                        nc.vector.tensor_add(x, x, bo_b)
                    st = stat_pool.tile([P, 6], f32, tag="st", name="st")
                    nc.vector.bn_stats(st, x)
                    mv = stat_pool.tile([P, 2], f32, tag="mv", name="mv")
                    nc.vector.bn_aggr(mv, st)
                    std = stat_pool.tile([P, 1], f32, tag="sd", name="std")
                    nc.scalar.activation(std, mv[:, 1:2], AF.Sqrt, bias=eps_t)
                    rstd = stat_pool.tile([P, 1], f32, tag="rs", name="rstd")
                    nc.vector.reciprocal(rstd, std)
                    y = resid_pool.tile([P, D], f32, tag="resid", name="y")
                    nc.vector.tensor_scalar(
                        out=y, in0=x, scalar1=mv[:, 0:1], scalar2=rstd,
                        op0=OP.subtract, op1=OP.mult)
                    if nonzero_affine:
                        nc.vector.tensor_mul(y, y, gam_b)
                        nc.vector.tensor_add(y, y, bet_b)
                    nc.sync.dma_start(out[m * P:(m + 1) * P, :], y)

            if repeat == 1:
                body()
            else:
                with tc.For_i(0, repeat, 1) as iv:
                    body(iv)

    nc.compile()
    return nc


def _pack_pairs_rows(a):
    """[K*128, C] -> [K//2, 128, 2, C] pair layout (k = 2*j + par, row = k*128+p)."""
    K = a.shape[0] // P
    return np.ascontiguousarray(
        a.reshape(K // 2, 2, P, a.shape[1]).transpose(0, 2, 1, 3))


def _prep_in_maps(inputs):
    f8 = ml_dtypes.float8_e4m3
    Q = np.asarray(inputs["Q"], np.float32)
    K = np.asarray(inputs["K"], np.float32)
    V = np.asarray(inputs["V"], np.float32)
    mask = np.asarray(inputs["attn_mask"])
    Wq = np.asarray(inputs["Wq"], np.float32)
    Wk = np.asarray(inputs["Wk"], np.float32)
    Wv = np.asarray(inputs["Wv"], np.float32)
    Wo = np.asarray(inputs["Wo"], np.float32)
    bq = np.asarray(inputs["bq"], np.float32)
    bk = np.asarray(inputs["bk"], np.float32)
    bv = np.asarray(inputs["bv"], np.float32)
    bo = np.asarray(inputs["bo"], np.float32)

    nonzero_bias = any(np.any(np.asarray(inputs[k])) for k in ("bq", "bk", "bv", "bo"))
    nonzero_affine = (np.any(np.asarray(inputs["gamma"]) != 1.0)
                      or np.any(np.asarray(inputs["beta"])))

    # Folded per-head weights: Wqk = SQ * Wq_h Wk_h^T, Wvo = SV * Wv_h Wo_h
    Wq_h = Wq.reshape(D, H, DH).transpose(1, 0, 2)       # [H, D, DH]
    Wk_h = Wk.reshape(D, H, DH).transpose(1, 0, 2)
    Wv_h = Wv.reshape(D, H, DH).transpose(1, 0, 2)
    Wo_h = Wo.reshape(H, DH, D)
    Wqk = SQ * np.einsum("had,hbd->hab", Wq_h, Wk_h)     # [H, D, D]
    Wvo = SV * np.einsum("had,hdb->hab", Wv_h, Wo_h)     # [H, D, D]
    wqk = np.stack([_pack_pairs_rows(Wqk[h]) for h in range(H)]).astype(f8)
    wvo = np.stack([_pack_pairs_rows(Wvo[h]) for h in range(H)]).astype(f8)

    if nonzero_bias:
        bq_h = bq.reshape(H, DH)
        bv_h = bv.reshape(H, DH)
        # out gets + (bv_h @ Wo_h) summed over heads, plus bo
        boeff = bo + np.einsum("hd,hdb->b", bv_h, Wo_h)
        # scores bias terms: the per-query (bk-derived) and constant terms
        # cancel in softmax; only the per-key term (from bq) survives.
        wk_bq = np.einsum("had,hd->ha", Wk_h, bq_h)      # [H, D]
        isq = 1.0 / float(np.sqrt(DH))

    # diag(-240) stationaries for the additive mask matmul (par 0 / par 1)
    diagm = np.zeros((2, P, 2, P), np.float32)
    for i in range(2):
        diagm[i, np.arange(P), i, np.arange(P)] = -240.0
    diagm = diagm.astype(f8)

    in_maps = []
    for b in range(B):
        m = {
            "xq": _pack_pairs_rows(np.ascontiguousarray(Q[b].T)).astype(f8),
            "xk": _pack_pairs_rows(np.ascontiguousarray(K[b].T)).astype(f8),
            "xv": _pack_pairs_rows(V[b]).astype(f8),
            "qres": np.ascontiguousarray(Q[b]),
            "wqk": wqk, "wvo": wvo,
        }
        masked = mask[b].astype(np.float32).T * 240.0     # [Sk, Sq], 240 = masked
        m["maskM"] = _pack_pairs_rows(masked).astype(f8)
        m["diagm"] = diagm
        if nonzero_bias:
            ebias = np.einsum("sd,hd->hs", K[b], wk_bq) * isq + EBIAS  # [H, Sk]
            m["ebias"] = ebias.astype(np.float32)
            m["boeff"] = boeff.astype(np.float32)
        if nonzero_affine:
            m["gam"] = np.asarray(inputs["gamma"], np.float32)
            m["bet"] = np.asarray(inputs["beta"], np.float32)
        in_maps.append(m)
    return in_maps, nonzero_bias, nonzero_affine


def kernel(**inputs):
    from concourse.bass_utils import run_bass_kernel_spmd

    in_maps, nzb, nza = _prep_in_maps(inputs)
    key = (1, nzb, nza)
    if key not in _cache:
        _cache[key] = build_nc(repeat=1, nonzero_bias=nzb, nonzero_affine=nza)
    nc = _cache[key]
    res = run_bass_kernel_spmd(nc, in_maps, list(range(B)))
    return np.stack([res.results[c]["out"] for c in range(B)], axis=0).astype(np.float32)


# revision 9
# speedup vs baseline: 1.1767x; 1.1767x over previous
"""Multi-head attention + residual + LayerNorm kernel for Trainium2 (8 NeuronCores).

Sharding: pure data parallel over batch (B=8 -> 1 batch element per core).
No collectives.

Key restructuring vs a direct implementation: per head, the QK and VO weight
pairs are folded on the host into single [D, D] matrices
    Wqk_h = Wq_h @ Wk_h^T   (scores = Q Wqk K^T, scale folded into exp)
    Wvo_h = Wv_h @ Wo_h     (out   = (attn @ V) Wvo)
which removes the K and V projections entirely and halves the contraction
depth of the scores and attention*V matmuls (512 instead of 1024). All heavy
matmuls run fp8e4 DoubleRow (K packed in pairs -> [128, 2, free] tiles) with
fp32 PSUM accumulation. The folded weights are scaled x64 (fp8 subnormal
avoidance); exp() folds in a x1/4 bias so E and E@V stay inside fp8e4m3
range (alpha cancels in the softmax ratio). The attention mask is applied
additively INSIDE the scores PSUM accumulation via one extra DoubleRow
matmul per psum: diag(-240) stationary x {0,240} mask tile adds -57600 to
masked scores, so exp() emits exact fp8 zeros and no post-exp elementwise
mask op exists at all (it was the critical-path serializer).

Softmax normalization is folded into the attn@V eviction: the denominator
row (every partition identical, from a ones-matmul with value 2^-6) is
reciprocal'd once per head and multiplied in during the PSUM->fp8 eviction.
That makes per-head outputs summable, so the output projection accumulates
over ALL heads inside PSUM in a tail phase (one residual-add per row tile).

Per-core dataflow per head h (Sq = query positions, Sk = key positions):
  tT [Dc,Sq]   = Wqk_h^T @ Qb^T          (DR fp8; evict fp8 pairs, ACT)
  ST [Sk,Sq]   = Kb^T-chunks^T @ tT - 57600*mask   (DR fp8, transposed)
  E            = exp(ST/2048 + ln 1/4)   (ACT from PSUM, fp8 pair tiles)
  den[128,Sq]  = (2^-6 ones)^T @ E ; recip = 1/den   (DVE, from PSUM)
  cxv[Dc,Sq]   = (Vb^T-chunks^T @ E) * recip         (DR fp8 + DVE evict)
tail:
  outp[Sq,D]   = sum_h cxv_h^T @ Wvo_h   (PSUM accumulation over heads)
  acc          = outp/4096 + Q           (DVE STT)
  LayerNorm over D per row (bn_stats/bn_aggr + sqrt + reciprocal).

The per-head stages are software-pipelined (head h+1's tT projection issues
between head h's scores and attention tail) to keep the PE array busy.
"""

import sys

sys.path.insert(0, "/opt/trn_rl_repo")

import numpy as np
import ml_dtypes

B, S, D, H = 8, 1024, 512, 8
DH = 2 * D            # per-head dim (module uses d_model*2 per head)
P = 128               # SBUF partitions
NS = S // 512         # 512-wide free-dim chunks over sequence (2)
MT = S // P           # 128-partition tiles over sequence (8)
KD = D // P           # 128-chunks over d_model (4)
JD = KD // 2          # DoubleRow K-pairs over d_model (2)
JS = MT // 2          # DoubleRow K-pairs over sequence (4)
SQ = 64.0             # host scale on Wqk (fp8 subnormal avoidance)
SV = 64.0             # host scale on Wvo
OMEGA = 2.0 ** -6     # ones value in den matmul (min normal fp8) -> cxv = ctx/OMEGA
PSC = 1.0 / (SV / OMEGA)                # tail un-scale: 1/4096
ESC = 1.0 / (SQ * float(np.sqrt(DH)))   # exp input scale
EBIAS = float(np.log(0.25))             # exp bias: E *= 1/4 (fp8 range)

_cache = {}


def build_nc(repeat=1, nonzero_bias=False, nonzero_affine=False):
    """Build the per-core Bass program. All 8 cores run this SPMD."""
    import concourse.bass as bass
    import concourse.tile as tile
    from concourse import bacc, mybir

    f32 = mybir.dt.float32
    f8 = mybir.dt.float8e4
    AF = mybir.ActivationFunctionType
    OP = mybir.AluOpType
    DR = mybir.MatmulPerfMode.DoubleRow

    nc = bacc.Bacc("TRN2", target_bir_lowering=False, debug=False, num_devices=8)

    # DRAM I/O (per core). Pair layouts: [j, 128(p), 2(par), cols] where the
    # contraction index is k = 2*j + par, row = k*128 + p.
    xq_d = nc.dram_tensor("xq", [JD, P, 2, S], f8, kind="ExternalInput").ap()
    xk_d = nc.dram_tensor("xk", [JD, P, 2, S], f8, kind="ExternalInput").ap()
    xv_d = nc.dram_tensor("xv", [JS, P, 2, D], f8, kind="ExternalInput").ap()
    qres = nc.dram_tensor("qres", [S, D], f32, kind="ExternalInput").ap()
    maskM = nc.dram_tensor("maskM", [JS, P, 2, S], f8, kind="ExternalInput").ap()
    diag_d = nc.dram_tensor("diagm", [2, P, 2, P], f8, kind="ExternalInput").ap()
    wqk_d = nc.dram_tensor("wqk", [H, JD, P, 2, D], f8, kind="ExternalInput").ap()
    wvo_d = nc.dram_tensor("wvo", [H, JD, P, 2, D], f8, kind="ExternalInput").ap()
    if nonzero_bias:
        ebias_d = nc.dram_tensor("ebias", [H, S], f32, kind="ExternalInput").ap()
        boeff = nc.dram_tensor("boeff", [D], f32, kind="ExternalInput").ap()
    if nonzero_affine:
        gam = nc.dram_tensor("gam", [D], f32, kind="ExternalInput").ap()
        bet = nc.dram_tensor("bet", [D], f32, kind="ExternalInput").ap()
    out = nc.dram_tensor("out", [S, D], f32, kind="ExternalOutput").ap()

    def bcast_ap(src_1d, n):
        return bass.AP(tensor=src_1d.tensor, offset=src_1d.offset,
                       ap=[[0, P]] + list(src_1d.ap))

    with tile.TileContext(nc) as tc:
        import contextlib
        with contextlib.ExitStack() as ctx:
            const = ctx.enter_context(tc.tile_pool(name="const", bufs=1))
            persist = ctx.enter_context(tc.tile_pool(name="persist", bufs=1))
            wpool = ctx.enter_context(tc.tile_pool(name="wpool", bufs=4))
            wvopool = ctx.enter_context(tc.tile_pool(name="wvop", bufs=2 * H))
            tt_pool = ctx.enter_context(tc.tile_pool(name="tt", bufs=4))
            e_pool = ctx.enter_context(tc.tile_pool(name="ee", bufs=8))
            cx_pool = ctx.enter_context(tc.tile_pool(name="cx", bufs=2 * H))
            den_pool = ctx.enter_context(tc.tile_pool(name="den", bufs=2))
            resid_pool = ctx.enter_context(tc.tile_pool(name="resid", bufs=8))
            stat_pool = ctx.enter_context(tc.tile_pool(name="stat", bufs=8))
            mpool = ctx.enter_context(tc.tile_pool(name="mask", bufs=JS))
            mm_psum = ctx.enter_context(tc.tile_pool(name="mmps", bufs=3, space="PSUM"))
            sm_psum = ctx.enter_context(tc.tile_pool(name="smps", bufs=2, space="PSUM"))

            ones_pair = const.tile([P, 2, P], f8)
            nc.vector.memset(ones_pair, OMEGA)
            diag_par = [const.tile([P, 2, P], f8, tag=f"dg{i}", name=f"dg{i}")
                        for i in range(2)]
            for i in range(2):
                nc.sync.dma_start(diag_par[i], diag_d[i])
            eps_t = const.tile([P, 1], f32)
            nc.vector.memset(eps_t, 1e-5)
            ebias_t = const.tile([P, 1], f32)
            nc.vector.memset(ebias_t, EBIAS)
            psc_t = const.tile([P, 1], f32)
            nc.vector.memset(psc_t, PSC)

            if nonzero_bias:
                ebias_sb = const.tile([P, H * MT], f32)
                nc.sync.dma_start(ebias_sb, ebias_d.rearrange("h (c p) -> p (h c)", p=P))
                bo_b = const.tile([P, D], f32)
                nc.sync.dma_start(bo_b, bcast_ap(boeff, D))
            if nonzero_affine:
                gam_b = const.tile([P, D], f32)
                nc.sync.dma_start(gam_b, bcast_ap(gam, D))
                bet_b = const.tile([P, D], f32)
                nc.sync.dma_start(bet_b, bcast_ap(bet, D))

            # Persistent SBUF inputs (fp8 pair tiles)
            xq_sb = [persist.tile([P, 2, S], f8, tag=f"xq{j}", name=f"xq{j}") for j in range(JD)]
            xk_sb = [persist.tile([P, 2, S], f8, tag=f"xk{j}", name=f"xk{j}") for j in range(JD)]
            xv_sb = [persist.tile([P, 2, D], f8, tag=f"xv{j}", name=f"xv{j}") for j in range(JS)]

            def body(iv=None):
                wqk0 = [wpool.tile([P, 2, D], f8, tag="w", name="wqk0") for _ in range(JD)]
                for j in range(JD):
                    nc.sync.dma_start(xq_sb[j], xq_d[j])
                    nc.sync.dma_start(wqk0[j], wqk_d[0, j])
                for j in range(JD):
                    nc.sync.dma_start(xk_sb[j], xk_d[j])
                for j in range(JS):
                    nc.sync.dma_start(xv_sb[j], xv_d[j])
                mask_sb = [mpool.tile([P, 2, S], f8, tag="mk", name="mk") for _ in range(JS)]
                for j in range(JS):
                    nc.sync.dma_start(mask_sb[j], maskM[j])
                qres_sb = [resid_pool.tile([P, D], f32, tag="resid", name="qres_sb")
                           for _ in range(MT)]
                for m in range(MT):
                    nc.sync.dma_start(qres_sb[m], qres[m * P:(m + 1) * P, :])

                # ---- per-head stages (software-pipelined: A(h+1) issues
                # between S(h) and B(h) so the PE has independent work while
                # head h's exp/mask evictions drain)
                def stage_A(h, wqk_t):
                    """tT projection: [Dc, Sq] fp8 pairs."""
                    tt = [tt_pool.tile([P, 2, S], f8, tag="tt", name="tt") for _ in range(JD)]
                    for c in range(KD):
                        ps = mm_psum.tile([P, S], f32, tag="mm", name="ps")
                        for j in range(JD):
                            for n in range(NS):
                                nc.tensor.matmul(
                                    ps[:, n * 512:(n + 1) * 512],
                                    lhsT=wqk_t[j][:, :, c * P:(c + 1) * P],
                                    rhs=xq_sb[j][:, :, n * 512:(n + 1) * 512],
                                    start=(j == 0), stop=(j == JD - 1),
                                    perf_mode=DR, skip_group_check=True)
                        if c % 2 == 0:
                            nc.scalar.copy(tt[c // 2][:, c % 2, :], ps)
                        else:
                            nc.vector.tensor_copy(tt[c // 2][:, c % 2, :], ps)
                    return tt

                def stage_S(h, tt):
                    """Transposed scores (+additive mask) -> exp -> E fp8."""
                    est = [e_pool.tile([P, 2, S], f8, tag="e", name="est") for _ in range(JS)]
                    for m in range(MT):
                        ps = mm_psum.tile([P, S], f32, tag="mm", name="ps")
                        for j in range(JD):
                            for n in range(NS):
                                nc.tensor.matmul(
                                    ps[:, n * 512:(n + 1) * 512],
                                    lhsT=xk_sb[j][:, :, m * P:(m + 1) * P],
                                    rhs=tt[j][:, :, n * 512:(n + 1) * 512],
                                    start=(j == 0), stop=False,
                                    perf_mode=DR, skip_group_check=True)
                        for n in range(NS):
                            nc.tensor.matmul(
                                ps[:, n * 512:(n + 1) * 512],
                                lhsT=diag_par[m % 2],
                                rhs=mask_sb[m // 2][:, :, n * 512:(n + 1) * 512],
                                start=False, stop=(n == NS - 1),
                                perf_mode=DR, skip_group_check=True)
                        if nonzero_bias:
                            bcol = ebias_sb[:, h * MT + m:h * MT + m + 1]
                        else:
                            bcol = ebias_t
                        nc.scalar.activation(est[m // 2][:, m % 2, :], ps,
                                             AF.Exp, bias=bcol, scale=ESC)
                    return est

                def stage_B(h, est, cxv):
                    """den + recip, attn@V with fused normalization. The den
                    matmuls interleave into ctxVT chunk 0 so the est[3]
                    (exp m6/m7) consumers issue as late as possible."""
                    def ctx_mm(ps, c, j):
                        for n in range(NS):
                            nc.tensor.matmul(
                                ps[:, n * 512:(n + 1) * 512],
                                lhsT=xv_sb[j][:, :, c * P:(c + 1) * P],
                                rhs=est[j][:, :, n * 512:(n + 1) * 512],
                                start=(j == 0), stop=(j == JS - 1),
                                perf_mode=DR, skip_group_check=True)

                    ps0 = mm_psum.tile([P, S], f32, tag="mm", name="ps")
                    for j in range(JS - 1):
                        ctx_mm(ps0, 0, j)
                    psd = mm_psum.tile([P, S], f32, tag="mm", name="ps")
                    for j in range(JS):
                        for n in range(NS):
                            nc.tensor.matmul(
                                psd[:, n * 512:(n + 1) * 512], lhsT=ones_pair,
                                rhs=est[j][:, :, n * 512:(n + 1) * 512],
                                start=(j == 0), stop=(j == JS - 1),
                                perf_mode=DR, skip_group_check=True)
                    ctx_mm(ps0, 0, JS - 1)
                    recip = den_pool.tile([P, S], f32, tag="den", name="recip")
                    nc.vector.reciprocal(recip, psd)
                    nc.vector.tensor_tensor(out=cxv[0][:, 0, :],
                                            in0=ps0, in1=recip, op=OP.mult)

                    for c in range(1, KD):
                        ps = mm_psum.tile([P, S], f32, tag="mm", name="ps")
                        for j in range(JS):
                            ctx_mm(ps, c, j)
                        nc.vector.tensor_tensor(out=cxv[c // 2][:, c % 2, :],
                                                in0=ps, in1=recip, op=OP.mult)

                # pipeline: A(0), then per h: S(h), [prefetch + A(h+1)], B(h)
                wqk_t = wqk0
                cxv_all = []
                wvo_all = []
                tt = stage_A(0, wqk_t)
                for h in range(H):
                    wvo_t = [wvopool.tile([P, 2, D], f8, tag="wv", name="wvo_t")
                             for _ in range(JD)]
                    for j in range(JD):
                        nc.sync.dma_start(wvo_t[j], wvo_d[h, j])
                    wvo_all.append(wvo_t)
                    est = stage_S(h, tt)
                    if h + 1 < H:
                        wqk_n = [wpool.tile([P, 2, D], f8, tag="w", name="wqk_n")
                                 for _ in range(JD)]
                        for j in range(JD):
                            nc.sync.dma_start(wqk_n[j], wqk_d[h + 1, j])
                        tt = stage_A(h + 1, wqk_n)
                    cxv = [cx_pool.tile([P, 2, S], f8, tag="cx", name="cxv")
                           for _ in range(JD)]
                    cxv_all.append(cxv)
                    stage_B(h, est, cxv)

                # ---- tail: output projection accumulated over all heads,
                # then residual add + LayerNorm + store
                for m in range(MT):
                    ps2 = sm_psum.tile([P, D], f32, tag="sm", name="ops")
                    for h in range(H):
                        for j in range(JD):
                            nc.tensor.matmul(
                                ps2, lhsT=cxv_all[h][j][:, :, m * P:(m + 1) * P],
                                rhs=wvo_all[h][j],
                                start=(h == 0 and j == 0),
                                stop=(h == H - 1 and j == JD - 1),
                                perf_mode=DR, skip_group_check=True)
                    x = resid_pool.tile([P, D], f32, tag="acc", bufs=8, name="acc")
                    nc.vector.scalar_tensor_tensor(
                        out=x, in0=ps2, scalar=psc_t, in1=qres_sb[m],
                        op0=OP.mult, op1=OP.add)
                    if nonzero_bias:
                        nc.vector.tensor_add(x, x, bo_b)
                    st = stat_pool.tile([P, 6], f32, tag="st", name="st")
                    nc.vector.bn_stats(st, x)
                    mv = stat_pool.tile([P, 2], f32, tag="mv", name="mv")
                    nc.vector.bn_aggr(mv, st)
                    std = stat_pool.tile([P, 1], f32, tag="sd", name="std")
                    nc.scalar.activation(std, mv[:, 1:2], AF.Sqrt, bias=eps_t)
                    rstd = stat_pool.tile([P, 1], f32, tag="rs", name="rstd")
                    nc.vector.reciprocal(rstd, std)
                    y = resid_pool.tile([P, D], f32, tag="resid", name="y")
                    nc.vector.tensor_scalar(
                        out=y, in0=x, scalar1=mv[:, 0:1], scalar2=rstd,
                        op0=OP.subtract, op1=OP.mult)
                    if nonzero_affine:
                        nc.vector.tensor_mul(y, y, gam_b)
                        nc.vector.tensor_add(y, y, bet_b)
                    nc.sync.dma_start(out[m * P:(m + 1) * P, :], y)

            if repeat == 1:
                body()
            else:
                with tc.For_i(0, repeat, 1) as iv:
                    body(iv)

    nc.compile()
    return nc


def _pack_pairs_rows(a):
    """[K*128, C] -> [K//2, 128, 2, C] pair layout (k = 2*j + par, row = k*128+p)."""
    K = a.shape[0] // P
    return np.ascontiguousarray(
        a.reshape(K // 2, 2, P, a.shape[1]).transpose(0, 2, 1, 3))


def _prep_in_maps(inputs):
    f8 = ml_dtypes.float8_e4m3
    Q = np.asarray(inputs["Q"], np.float32)
    K = np.asarray(inputs["K"], np.float32)
    V = np.asarray(inputs["V"], np.float32)
    mask = np.asarray(inputs["attn_mask"])
    Wq = np.asarray(inputs["Wq"], np.float32)
    Wk = np.asarray(inputs["Wk"], np.float32)
    Wv = np.asarray(inputs["Wv"], np.float32)
    Wo = np.asarray(inputs["Wo"], np.float32)
    bq = np.asarray(inputs["bq"], np.float32)
    bk = np.asarray(inputs["bk"], np.float32)
    bv = np.asarray(inputs["bv"], np.float32)
    bo = np.asarray(inputs["bo"], np.float32)

    nonzero_bias = any(np.any(np.asarray(inputs[k])) for k in ("bq", "bk", "bv", "bo"))
    nonzero_affine = (np.any(np.asarray(inputs["gamma"]) != 1.0)
                      or np.any(np.asarray(inputs["beta"])))

    # Folded per-head weights: Wqk = SQ * Wq_h Wk_h^T, Wvo = SV * Wv_h Wo_h
    Wq_h = Wq.reshape(D, H, DH).transpose(1, 0, 2)       # [H, D, DH]
    Wk_h = Wk.reshape(D, H, DH).transpose(1, 0, 2)
    Wv_h = Wv.reshape(D, H, DH).transpose(1, 0, 2)
    Wo_h = Wo.reshape(H, DH, D)
    Wqk = SQ * np.einsum("had,hbd->hab", Wq_h, Wk_h)     # [H, D, D]
    Wvo = SV * np.einsum("had,hdb->hab", Wv_h, Wo_h)     # [H, D, D]
    wqk = np.stack([_pack_pairs_rows(Wqk[h]) for h in range(H)]).astype(f8)
    wvo = np.stack([_pack_pairs_rows(Wvo[h]) for h in range(H)]).astype(f8)

    if nonzero_bias:
        bq_h = bq.reshape(H, DH)
        bv_h = bv.reshape(H, DH)
        # out gets + (bv_h @ Wo_h) summed over heads, plus bo
        boeff = bo + np.einsum("hd,hdb->b", bv_h, Wo_h)
        # scores bias terms: the per-query (bk-derived) and constant terms
        # cancel in softmax; only the per-key term (from bq) survives.
        wk_bq = np.einsum("had,hd->ha", Wk_h, bq_h)      # [H, D]
        isq = 1.0 / float(np.sqrt(DH))

    # diag(-240) stationaries for the additive mask matmul (par 0 / par 1)
    diagm = np.zeros((2, P, 2, P), np.float32)
    for i in range(2):
        diagm[i, np.arange(P), i, np.arange(P)] = -240.0
    diagm = diagm.astype(f8)

    in_maps = []
    for b in range(B):
        m = {
            "xq": _pack_pairs_rows(np.ascontiguousarray(Q[b].T)).astype(f8),
            "xk": _pack_pairs_rows(np.ascontiguousarray(K[b].T)).astype(f8),
            "xv": _pack_pairs_rows(V[b]).astype(f8),
            "qres": np.ascontiguousarray(Q[b]),
            "wqk": wqk, "wvo": wvo,
        }
        masked = mask[b].astype(np.float32).T * 240.0     # [Sk, Sq], 240 = masked
        m["maskM"] = _pack_pairs_rows(masked).astype(f8)
        m["diagm"] = diagm
        if nonzero_bias:
            ebias = np.einsum("sd,hd->hs", K[b], wk_bq) * isq + EBIAS  # [H, Sk]
            m["ebias"] = ebias.astype(np.float32)
            m["boeff"] = boeff.astype(np.float32)
        if nonzero_affine:
            m["gam"] = np.asarray(inputs["gamma"], np.float32)
            m["bet"] = np.asarray(inputs["beta"], np.float32)
        in_maps.append(m)
    return in_maps, nonzero_bias, nonzero_affine


def kernel(**inputs):
    from concourse.bass_utils import run_bass_kernel_spmd

    in_maps, nzb, nza = _prep_in_maps(inputs)
    key = (1, nzb, nza)
    if key not in _cache:
        _cache[key] = build_nc(repeat=1, nonzero_bias=nzb, nonzero_affine=nza)
    nc = _cache[key]
    res = run_bass_kernel_spmd(nc, in_maps, list(range(B)))
    return np.stack([res.results[c]["out"] for c in range(B)], axis=0).astype(np.float32)
